# revision 27
# baseline (speedup 1.0000x reference)
"""CLIP attention (B=8, S=1024, H=1024, 16 heads) on 8 TRN2 NeuronCores.

Sharding: data-parallel over batch — core b computes attention for x[b].

v4 design (vs v2/v3):
  - bf16 everywhere on the PE: x -> xT via f32r PE transpose with cast on
    evac; Wq/Wk per-pair column-chunks + Wv row-chunks DMA'd f32 and
    DVE-cast to bf16 (strided byte-slice DMA measured 135us/chunk — dead
    end); Wo cast on the otherwise-idle GPSIMD
  - ACT (exp) is the attention bottleneck: 128 x [128,1024] exps = 147us.
    The schedule keeps ACT dense: pair 0 runs ALL 16 scores+exp units
    first (pt pool holds them; u deferred) while V-proj + pair-1's Q/K
    projection run on the PE underneath, then pair-0's u batch
  - steady pairs: flat 16-slot loop, u lagged 2 slots behind exp so the
    qh boundary doesn't stall the next scores (was ~1us ACT gap x3/pair)
  - normalization: reciprocal_approx_fast (vector.reciprocal was 6.5us
    per [128,S] and blocked evacs in the DVE FIFO ~4.4us/pair)
  - DMA priority: ident, x, pair-0 Wq/Wk cols, small biases, Wv, then
    per-pair Wq/Wk cols + Wo rows inside the loop
"""

import numpy as np

B = 8
S = 1024
H = 1024
NH = 16
D = 64
P = 128
NT = 8          # number of 128-tiles along S or H
SCALE = 0.125   # 1/sqrt(64)

_CACHE = {}


def _build():
    import concourse.bacc as bacc
    import concourse.mybir as mybir
    import concourse.tile as tile
    from contextlib import ExitStack

    F32 = mybir.dt.float32
    F32R = mybir.dt.float32r
    BF16 = mybir.dt.bfloat16
    EXP = mybir.ActivationFunctionType.Exp

    nc = bacc.Bacc(None)
    x = nc.dram_tensor("x", [S, H], F32, kind="ExternalInput")
    wq = nc.dram_tensor("Wq", [H, H], F32, kind="ExternalInput")
    wk = nc.dram_tensor("Wk", [H, H], F32, kind="ExternalInput")
    wv = nc.dram_tensor("Wv", [H, H], F32, kind="ExternalInput")
    wo = nc.dram_tensor("Wo", [H, H], F32, kind="ExternalInput")
    bq = nc.dram_tensor("bq", [H], F32, kind="ExternalInput")
    bk = nc.dram_tensor("bk", [H], F32, kind="ExternalInput")
    bv = nc.dram_tensor("bv", [H], F32, kind="ExternalInput")
    bo = nc.dram_tensor("bo", [H], F32, kind="ExternalInput")
    ident = nc.dram_tensor("ident", [P, P], F32, kind="ExternalInput")
    out = nc.dram_tensor("out", [S, H], F32, kind="ExternalOutput")
    rscr = nc.dram_tensor("rscr", [NH, S], F32)   # scratch for r broadcast

    with tile.TileContext(nc) as tc, ExitStack() as ctx:
        pers = ctx.enter_context(tc.tile_pool(name="pers", bufs=1))
        xT = pers.tile([P, NT, S], BF16, name="xT")
        vp = pers.tile([P, NT, NH * (D + 1)], BF16, name="vp")
        mergedT = pers.tile([P, NT, S], BF16, name="mergedT")
        wo_sb = pers.tile([P, NT, H], BF16, name="wo_sb")
        wv_sb = pers.tile([P, NT, H], BF16, name="wv_sb")

        small = ctx.enter_context(tc.tile_pool(name="small", bufs=1))
        bq_sb = small.tile([P, NT], F32, name="bq_sb")
        bk_sb = small.tile([P, NT], F32, name="bk_sb")
        bv_bc = small.tile([P, H], F32, name="bv_bc")
        bo_bc = small.tile([P, H], F32, name="bo_bc")
        ones16 = small.tile([P, NH], F32, name="ones16")

        wqkstage = ctx.enter_context(tc.tile_pool(name="wqks", bufs=2))
        wqkpool = ctx.enter_context(tc.tile_pool(name="wqk", bufs=2))
        wostage = ctx.enter_context(tc.tile_pool(name="wost", bufs=2))
        wvstage = ctx.enter_context(tc.tile_pool(name="wvst", bufs=3))
        qkpool = ctx.enter_context(tc.tile_pool(name="qk", bufs=2))
        ptpool = ctx.enter_context(tc.tile_pool(name="ptp", bufs=16))
        rbpool = ctx.enter_context(tc.tile_pool(name="rb", bufs=1))
        rppool = ctx.enter_context(tc.tile_pool(name="rp", bufs=1))

        def load_wqk_stage(hp):
            """DMA the [H, 128] column slices of Wq/Wk for pair hp (f32)."""
            stgs = []
            for src, nm in ((wq, "q"), (wk, "k")):
                stg = wqkstage.tile([P, NT, P], F32, tag=f"ws{nm}",
                                    name=f"ws{nm}{hp}")
                nc.sync.dma_start(
                    stg[:],
                    src[:, P * hp:P * (hp + 1)]
                    .rearrange("(kk p) c -> p kk c", p=P))
                stgs.append(stg)
            return stgs

        def cast_wqk(hp, stgs):
            tiles = []
            for stg, nm in zip(stgs, ("q", "k")):
                t = wqkpool.tile([P, NT, P], BF16, tag=f"w{nm}c",
                                 name=f"w{nm}c{hp}")
                nc.vector.tensor_copy(t[:], stg[:])
                tiles.append(t)
            return tiles

        def load_wqk_cols(hp):
            return cast_wqk(hp, load_wqk_stage(hp))

        # ---- phase 0: DMAs + x -> xT via DMA-engine xbar transpose ----
        # x/Wv/Wo chunks split across two DMA rings — a single ring
        # serializes 512KB chunks at ~2.3us each. x -> bf16 cast (DVE)
        # then dma_start_transpose per chunk: no PE transposes (were
        # ~275ns x 64 = 17.6us of PE) and no evac copies.
        with tc.tile_pool(name="xstage", bufs=3) as xstage, \
             tc.tile_pool(name="xbstage", bufs=3) as xbstage:
            # (A) DMA issues only — keep DVE/ACT FIFOs clear
            xs_tiles = []
            for m in range(NT):
                xs = xstage.tile([P, H], F32, tag="xs", name=f"xs{m}")
                eng = nc.sync if m % 2 == 0 else nc.gpsimd
                eng.dma_start(xs[:], x[P * m:P * (m + 1), :])
                xs_tiles.append(xs)
            wqk0_stg = load_wqk_stage(0)
            wqk1_stg = load_wqk_stage(1)
            nc.sync.dma_start(bq_sb[:], bq.rearrange("(r p) -> p r", p=P))
            nc.sync.dma_start(bk_sb[:], bk.rearrange("(r p) -> p r", p=P))
            nc.sync.dma_start(bv_bc[:], bv[None, :].to_broadcast((P, H)))
            wv_stgs = []
            for kk in range(NT):
                ws = wvstage.tile([P, H], F32, tag="wvs", name=f"wvs{kk}")
                eng = nc.sync if kk % 2 == 0 else nc.gpsimd
                eng.dma_start(ws[:], wv[P * kk:P * (kk + 1), :])
                wv_stgs.append(ws)
            wo_stgs = []
            for kk in range(NT):
                wos = wostage.tile([P, H], F32, tag="wos", name=f"wos{kk}")
                eng = nc.sync if kk % 2 == 0 else nc.gpsimd
                eng.dma_start(wos[:], wo[P * kk:P * (kk + 1), :])
                wo_stgs.append(wos)
            nc.sync.dma_start(bo_bc[:], bo[None, :].to_broadcast((P, H)))

            # (B) x casts + DMA transposes into xT (HWDGE rings only)
            for m in range(NT):
                xsb = xbstage.tile([P, H], BF16, tag="xsb", name=f"xsb{m}")
                nc.vector.tensor_copy(xsb[:], xs_tiles[m][:])
                eng = nc.sync if m % 2 == 0 else nc.scalar
                eng.dma_start_transpose(xT[:, :, P * m:P * (m + 1)], xsb[:])

            # (C) weight casts, behind the x casts in the DVE FIFO
            nc.vector.memset(ones16[:], 1.0)
            wqk0 = cast_wqk(0, wqk0_stg)
            wqk1 = cast_wqk(1, wqk1_stg)
            for kk in range(NT):
                eng = nc.vector if kk % 2 == 0 else nc.scalar
                if kk % 2 == 0:
                    nc.vector.tensor_copy(wv_sb[:, kk, :], wv_stgs[kk][:])
                else:
                    nc.scalar.copy(wv_sb[:, kk, :], wv_stgs[kk][:])
            for kk in range(NT):
                nc.gpsimd.tensor_copy(wo_sb[:, kk, :], wo_stgs[kk][:])

        # ---- attention ----
        # PSUM budget is 8 banks: qk(2) + sp(4) + one of {vpsum(2), up(2)}
        # — vpsum's scope closes before upsum opens.
        with tc.tile_pool(name="qkpsum", bufs=2, space="PSUM") as qkpsum, \
             tc.tile_pool(name="spsum", bufs=2, space="PSUM") as spsum:

            def qk_proj_thunks(hp, w_cols):
                """Yield thunks: 32 matmuls + 4 bias evacs for pair hp."""
                qt_n = qkpool.tile([P, S], BF16, tag="qt", name=f"qt{hp}")
                kt_n = qkpool.tile([P, S], BF16, tag="kt", name=f"kt{hp}")
                thunks = []
                for w_t, dst, b_sb in ((w_cols[0], qt_n, bq_sb),
                                       (w_cols[1], kt_n, bk_sb)):
                    for n in range(2):
                        def group(w_t=w_t, dst=dst, b_sb=b_sb, n=n):
                            qps = qkpsum.tile([P, 512], F32, tag="qk",
                                              name=f"qk{hp}_{n}")
                            for kk in range(NT):
                                def mm(qps=qps, w_t=w_t, kk=kk, n=n):
                                    nc.tensor.matmul(
                                        qps[:],
                                        w_t[:, kk, :],
                                        xT[:, kk, 512 * n:512 * (n + 1)],
                                        start=(kk == 0), stop=(kk == NT - 1))
                                yield mm
                            def bias(qps=qps, dst=dst, b_sb=b_sb, n=n):
                                nc.vector.tensor_scalar_add(
                                    dst[:, 512 * n:512 * (n + 1)], qps[:],
                                    b_sb[:, hp:hp + 1])
                            yield bias
                        thunks.extend(group())
                return qt_n, kt_n, thunks

            def v_proj_half(vpsum, m, n):
                ps = vpsum.tile([P, 512], F32, tag="ppv", name=f"ppv{m}_{n}")
                for kk in range(NT):
                    nc.tensor.matmul(
                        ps[:],
                        xT[:, kk, P * m:P * (m + 1)],
                        wv_sb[:, kk, 512 * n:512 * (n + 1)],
                        start=(kk == 0), stop=(kk == NT - 1))
                vview = (vp[:, m, (D + 1) * 8 * n:(D + 1) * 8 * (n + 1)]
                         .rearrange("p (h d) -> p h d", d=D + 1))
                nc.vector.tensor_add(
                    vview[:, :, 0:D],
                    ps[:].rearrange("p (h d) -> p h d", d=D),
                    bv_bc[:, 512 * n:512 * (n + 1)]
                    .rearrange("p (h d) -> p h d", d=D))
                nc.vector.tensor_copy(vview[:, :, D:D + 1],
                                      ones16[:, 8 * n:8 * (n + 1)]
                                      .unsqueeze(2))

            def scores_exp(hp, qt_c, kt_c, qh, kk):
                sph = spsum.tile([P, 1024], F32, tag="sp",
                                 name=f"sp{hp}_{qh}_{kk}")
                nc.tensor.matmul(
                    sph[:, 0:512],
                    kt_c[0:D, P * kk:P * (kk + 1)],
                    qt_c[0:D, 512 * qh:512 * (qh + 1)],
                    start=True, stop=True)
                nc.tensor.matmul(
                    sph[:, 512:1024],
                    kt_c[D:P, P * kk:P * (kk + 1)],
                    qt_c[D:P, 512 * qh:512 * (qh + 1)],
                    start=True, stop=True)
                pth = ptpool.tile([P, 1024], BF16, tag="pt",
                                  name=f"pt{hp}_{qh}_{kk}")
                nc.scalar.activation(pth[:], sph[:], EXP, scale=SCALE)
                return pth

            def normalize(hp, r_e, r_o):
                nc.sync.dma_start(rscr[2 * hp:2 * hp + 1, :], r_e[:])
                nc.sync.dma_start(rscr[2 * hp + 1:2 * hp + 2, :], r_o[:])
                rb = rbpool.tile([P, S], F32, tag="rb", name=f"rb{hp}")
                rbi = rbpool.tile([P, S], F32, tag="rbi", name=f"rbi{hp}")
                nc.sync.dma_start(
                    rb[0:D, :],
                    rscr[2 * hp, :][None, :].to_broadcast((D, S)))
                nc.sync.dma_start(
                    rb[D:P, :],
                    rscr[2 * hp + 1, :][None, :].to_broadcast((D, S)))
                nc.vector.reciprocal_approx_fast(rbi[:], rb[:])
                nc.vector.tensor_mul(mergedT[:, hp, :], mergedT[:, hp, :],
                                     rbi[:])

            class Pair:
                def __init__(self, hp):
                    self.hp = hp
                    self.he, self.ho = 2 * hp, 2 * hp + 1
                    self.r_e = rppool.tile([1, S], F32, tag="rpe",
                                           name=f"rpe{hp}")
                    self.r_o = rppool.tile([1, S], F32, tag="rpo",
                                           name=f"rpo{hp}")
                    self.up = {}

                def alloc_up(self, qh):
                    self.up[qh] = (
                        upsum.tile([D + 1, 512], F32, tag="upe",
                                   name=f"upe{self.hp}_{qh}"),
                        upsum.tile([D + 1, 512], F32, tag="upo",
                                   name=f"upo{self.hp}_{qh}"))

                def u_mms(self, pt, kk, qh):
                    up_e, up_o = self.up[qh]
                    nc.tensor.matmul(
                        up_e[:],
                        vp[:, kk, (D + 1) * self.he:(D + 1) * (self.he + 1)],
                        pt[:, 0:512],
                        start=(kk == 0), stop=(kk == NT - 1))
                    nc.tensor.matmul(
                        up_o[:],
                        vp[:, kk, (D + 1) * self.ho:(D + 1) * (self.ho + 1)],
                        pt[:, 512:1024],
                        start=(kk == 0), stop=(kk == NT - 1))

                def evac(self, qh):
                    hp = self.hp
                    up_e, up_o = self.up[qh]
                    nc.vector.tensor_copy(
                        mergedT[0:D, hp, 512 * qh:512 * (qh + 1)],
                        up_e[0:D, :])
                    nc.vector.tensor_copy(
                        mergedT[D:P, hp, 512 * qh:512 * (qh + 1)],
                        up_o[0:D, :])
                    nc.vector.tensor_copy(
                        self.r_e[0:1, 512 * qh:512 * (qh + 1)],
                        up_e[D:D + 1, :])
                    nc.vector.tensor_copy(
                        self.r_o[0:1, 512 * qh:512 * (qh + 1)],
                        up_o[D:D + 1, :])

                def normalize(self):
                    normalize(self.hp, self.r_e, self.r_o)

            # -- pair 0: Q/K proj, then ALL scores+exps (u deferred) with
            # pair-1's proj dripped into the exp-paced stream; V-proj runs
            # after, then pair-0's u batch --
            qt_cur, kt_cur, th0 = qk_proj_thunks(0, wqk0)
            for t in th0:
                t()

            qt_nxt, kt_nxt, pending = qk_proj_thunks(1, wqk1)
            pending = list(pending)
            p0 = Pair(0)
            pts0 = []
            for qh in range(2):
                for kk in range(NT):
                    pts0.append(scores_exp(0, qt_cur, kt_cur, qh, kk))
                    for _ in range(3):
                        if pending:
                            pending.pop(0)()
            with tc.tile_pool(name="vpsum", bufs=2, space="PSUM") as vpsum:
                for m in range(NT):
                    for n in range(2):
                        v_proj_half(vpsum, m, n)
                        if pending:
                            pending.pop(0)()
            with tc.tile_pool(name="upsum", bufs=1, space="PSUM") as upsum_p:
                upsum = upsum_p
                for qh in range(2):
                    p0.alloc_up(qh)
                    for kk in range(NT):
                        p0.u_mms(pts0[qh * NT + kk], kk, qh)
                        if pending:
                            pending.pop(0)()
                    p0.evac(qh)
                while pending:
                    pending.pop(0)()
                p0.normalize()
                qt_cur, kt_cur = qt_nxt, kt_nxt

                # -- pairs 1..7: flat 16-slot loop, u lagged 2 slots; the
                # last 2 u units + evac + normalize of pair p carry into
                # pair p+1's first slots (no PE tail before next scores) --
                carry = None
                for hp in range(1, NT):
                    pending = []
                    if hp + 1 < NT:
                        w_cols = load_wqk_cols(hp + 1)
                        qt_nxt, kt_nxt, pending = qk_proj_thunks(
                            hp + 1, w_cols)
                        pending = list(pending)

                    pr = Pair(hp)
                    pr.alloc_up(0)
                    pts = []
                    for s in range(NT * 2):
                        qh, kk = divmod(s, NT)
                        pts.append(scores_exp(hp, qt_cur, kt_cur, qh, kk))
                        if carry is not None:
                            cpr, cpts = carry
                            if s == 0:
                                cpr.u_mms(cpts[14], NT - 2, 1)
                            elif s == 1:
                                cpr.u_mms(cpts[15], NT - 1, 1)
                                cpr.evac(1)
                                cpr.normalize()
                                carry = None
                        if s == 2 + NT:
                            pr.alloc_up(1)
                        if 2 <= s < 2 + 14:
                            lqh, lkk = divmod(s - 2, NT)
                            pr.u_mms(pts[s - 2], lkk, lqh)
                            if s - 2 == NT - 1:
                                pr.evac(0)
                        for _ in range(3):
                            if pending:
                                pending.pop(0)()
                    while pending:
                        pending.pop(0)()
                    if hp + 1 < NT:
                        carry = (pr, pts)
                        qt_cur, kt_cur = qt_nxt, kt_nxt
                    else:
                        pr.u_mms(pts[14], NT - 2, 1)
                        pr.u_mms(pts[15], NT - 1, 1)
                        pr.evac(1)
                        pr.normalize()

        # ---- output projection ----
        with tc.tile_pool(name="opsum", bufs=4, space="PSUM") as opsum, \
             tc.tile_pool(name="ostage", bufs=4) as ostage:
            for q in range(NT):
                for n in range(2):
                    ps = opsum.tile([P, 512], F32, tag="op", name=f"op{q}_{n}")
                    for r in range(NT):
                        nc.tensor.matmul(
                            ps[:],
                            mergedT[:, r, P * q:P * (q + 1)],
                            wo_sb[:, r, 512 * n:512 * (n + 1)],
                            start=(r == 0), stop=(r == NT - 1))
                    os_t = ostage.tile([P, 512], F32, tag="os",
                                       name=f"os{q}_{n}")
                    nc.vector.tensor_add(os_t[:], ps[:],
                                         bo_bc[:, 512 * n:512 * (n + 1)])
                    nc.sync.dma_start(
                        out[P * q:P * (q + 1), 512 * n:512 * (n + 1)], os_t[:])

    nc.finalize()
    return nc


def _in_maps(inputs):
    x = np.ascontiguousarray(np.asarray(inputs["x"], dtype=np.float32))
    eye = np.eye(P, dtype=np.float32)
    common = {k: np.ascontiguousarray(np.asarray(inputs[k], dtype=np.float32))
              for k in ("Wq", "Wk", "Wv", "Wo", "bq", "bk", "bv", "bo")}
    return [{"x": x[b], "ident": eye, **common} for b in range(B)]


def _gather(res, inputs):
    return np.stack([res.results[b]["out"] for b in range(B)]).astype(np.float32)


def kernel(**inputs):
    from concourse.bass_utils import run_bass_kernel_spmd

    nc = _CACHE.get("nc")
    if nc is None:
        nc = _CACHE["nc"] = _build()

    in_maps = _in_maps(inputs)
    res = run_bass_kernel_spmd(nc, in_maps, list(range(B)))
    return _gather(res, inputs)


# revision 34
# speedup vs baseline: 1.0944x; 1.0944x over previous
"""CLIP attention (B=8, S=1024, H=1024, 16 heads) on 8 TRN2 NeuronCores.

Sharding: data-parallel over batch — core b computes attention for x[b].

v4 design (vs v2/v3):
  - bf16 everywhere on the PE: x -> xT via f32r PE transpose with cast on
    evac; Wq/Wk per-pair column-chunks + Wv row-chunks DMA'd f32 and
    DVE-cast to bf16 (strided byte-slice DMA measured 135us/chunk — dead
    end); Wo cast on the otherwise-idle GPSIMD
  - ACT (exp) is the attention bottleneck: 128 x [128,1024] exps = 147us.
    The schedule keeps ACT dense: pair 0 runs ALL 16 scores+exp units
    first (pt pool holds them; u deferred) while V-proj + pair-1's Q/K
    projection run on the PE underneath, then pair-0's u batch
  - steady pairs: flat 16-slot loop, u lagged 2 slots behind exp so the
    qh boundary doesn't stall the next scores (was ~1us ACT gap x3/pair)
  - normalization: reciprocal_approx_fast (vector.reciprocal was 6.5us
    per [128,S] and blocked evacs in the DVE FIFO ~4.4us/pair)
  - DMA priority: ident, x, pair-0 Wq/Wk cols, small biases, Wv, then
    per-pair Wq/Wk cols + Wo rows inside the loop
"""

import numpy as np

B = 8
S = 1024
H = 1024
NH = 16
D = 64
P = 128
NT = 8          # number of 128-tiles along S or H
SCALE = 0.125   # 1/sqrt(64)

_CACHE = {}


def _build():
    import concourse.bacc as bacc
    import concourse.mybir as mybir
    import concourse.tile as tile
    from contextlib import ExitStack

    F32 = mybir.dt.float32
    F32R = mybir.dt.float32r
    BF16 = mybir.dt.bfloat16
    EXP = mybir.ActivationFunctionType.Exp

    nc = bacc.Bacc(None)
    x = nc.dram_tensor("x", [S, H], F32, kind="ExternalInput")
    wq = nc.dram_tensor("Wq", [H, H], F32, kind="ExternalInput")
    wk = nc.dram_tensor("Wk", [H, H], F32, kind="ExternalInput")
    wv = nc.dram_tensor("Wv", [H, H], F32, kind="ExternalInput")
    wo = nc.dram_tensor("Wo", [H, H], F32, kind="ExternalInput")
    bq = nc.dram_tensor("bq", [H], F32, kind="ExternalInput")
    bk = nc.dram_tensor("bk", [H], F32, kind="ExternalInput")
    bv = nc.dram_tensor("bv", [H], F32, kind="ExternalInput")
    bo = nc.dram_tensor("bo", [H], F32, kind="ExternalInput")
    ident = nc.dram_tensor("ident", [P, P], F32, kind="ExternalInput")
    out = nc.dram_tensor("out", [S, H], F32, kind="ExternalOutput")
    rscr = nc.dram_tensor("rscr", [NH, S], F32)   # scratch for r broadcast

    with tile.TileContext(nc) as tc, ExitStack() as ctx:
        pers = ctx.enter_context(tc.tile_pool(name="pers", bufs=1))
        xT = pers.tile([P, NT, S], BF16, name="xT")
        vp = pers.tile([P, NT, NH * (D + 1)], BF16, name="vp")
        mergedT = pers.tile([P, NT, S], BF16, name="mergedT")
        wo_sb = pers.tile([P, NT, H], BF16, name="wo_sb")
        wv_sb = pers.tile([P, NT, H], BF16, name="wv_sb")

        small = ctx.enter_context(tc.tile_pool(name="small", bufs=1))
        bq_sb = small.tile([P, NT], F32, name="bq_sb")
        bk_sb = small.tile([P, NT], F32, name="bk_sb")
        bv_bc = small.tile([P, H], F32, name="bv_bc")
        bo_bc = small.tile([P, H], F32, name="bo_bc")
        ones16 = small.tile([P, NH], F32, name="ones16")

        wqkstage = ctx.enter_context(tc.tile_pool(name="wqks", bufs=2))
        wqkpool = ctx.enter_context(tc.tile_pool(name="wqk", bufs=2))
        wostage = ctx.enter_context(tc.tile_pool(name="wost", bufs=2))
        wvstage = ctx.enter_context(tc.tile_pool(name="wvst", bufs=3))
        qkpool = ctx.enter_context(tc.tile_pool(name="qk", bufs=2))
        ptpool = ctx.enter_context(tc.tile_pool(name="ptp", bufs=16))
        rbpool = ctx.enter_context(tc.tile_pool(name="rb", bufs=1))
        rppool = ctx.enter_context(tc.tile_pool(name="rp", bufs=1))

        def load_wqk_stage(hp):
            """DMA the [H, 128] column slices of Wq/Wk for pair hp (f32)."""
            stgs = []
            for src, nm in ((wq, "q"), (wk, "k")):
                stg = wqkstage.tile([P, NT, P], F32, tag=f"ws{nm}",
                                    name=f"ws{nm}{hp}")
                nc.sync.dma_start(
                    stg[:],
                    src[:, P * hp:P * (hp + 1)]
                    .rearrange("(kk p) c -> p kk c", p=P))
                stgs.append(stg)
            return stgs

        def cast_wqk(hp, stgs):
            tiles = []
            for stg, nm in zip(stgs, ("q", "k")):
                t = wqkpool.tile([P, NT, P], BF16, tag=f"w{nm}c",
                                 name=f"w{nm}c{hp}")
                nc.vector.tensor_copy(t[:], stg[:])
                tiles.append(t)
            return tiles

        def load_wqk_cols(hp):
            return cast_wqk(hp, load_wqk_stage(hp))

        # ---- phase 0: DMAs + x transposes ----
        # x split across both HWDGE rings (sync+scalar) — one ring
        # serializes 512KB chunks at ~2.3us each. Emission order keeps the
        # DVE FIFO clear: evacs first, then wq/wk casts; wv casts are
        # emitted AFTER pair-0's scores (only v_proj needs them) so they
        # can't head-of-line-block pair-0's bias evacs.
        with tc.tile_pool(name="xstage", bufs=3) as xstage, \
             tc.tile_pool(name="idpool", bufs=1) as idpool, \
             tc.tile_pool(name="tpsum", bufs=4, space="PSUM") as tpsum:
            identity = idpool.tile([P, P], F32R, name="identity")
            nc.sync.dma_start(identity[:], ident[:, :].bitcast(F32R))

            # (A) DMA issues only
            xs_tiles = []
            for m in range(NT):
                xs = xstage.tile([P, H], F32R, tag="xs", name=f"xs{m}")
                eng = nc.sync if m % 2 == 0 else nc.scalar
                eng.dma_start(xs[:], x[P * m:P * (m + 1), :].bitcast(F32R))
                xs_tiles.append(xs)
            wqk0_stg = load_wqk_stage(0)
            wqk1_stg = load_wqk_stage(1)
            nc.sync.dma_start(bq_sb[:], bq.rearrange("(r p) -> p r", p=P))
            nc.sync.dma_start(bk_sb[:], bk.rearrange("(r p) -> p r", p=P))
            nc.sync.dma_start(bv_bc[:], bv[None, :].to_broadcast((P, H)))
            wv_stgs = []
            for kk in range(NT):
                ws = wvstage.tile([P, H], F32, tag="wvs", name=f"wvs{kk}")
                eng = nc.sync if kk % 2 == 0 else nc.gpsimd
                eng.dma_start(ws[:], wv[P * kk:P * (kk + 1), :])
                wv_stgs.append(ws)
            wo_stgs = []
            for kk in range(NT):
                wos = wostage.tile([P, H], F32, tag="wos", name=f"wos{kk}")
                eng = nc.sync if kk % 2 == 0 else nc.gpsimd
                eng.dma_start(wos[:], wo[P * kk:P * (kk + 1), :])
                wo_stgs.append(wos)
            nc.sync.dma_start(bo_bc[:], bo[None, :].to_broadcast((P, H)))

            # (B) x -> xT PE transposes, evacs alternating DVE/ACT
            for m in range(NT):
                xs = xs_tiles[m]
                for r in range(NT):
                    tp = tpsum.tile([P, P], F32R, tag="tp", name=f"tp{m}_{r}")
                    nc.tensor.transpose(tp[:], xs[:, P * r:P * (r + 1)],
                                        identity[:])
                    if r % 2 == 0:
                        nc.vector.tensor_copy(xT[:, r, P * m:P * (m + 1)],
                                              tp[:].bitcast(F32))
                    else:
                        nc.scalar.copy(xT[:, r, P * m:P * (m + 1)],
                                       tp[:].bitcast(F32))

            # (C) Wq/Wk casts only (behind evacs in DVE FIFO); Wo on GPSIMD
            nc.vector.memset(ones16[:], 1.0)
            wqk0 = cast_wqk(0, wqk0_stg)
            wqk1 = cast_wqk(1, wqk1_stg)
            for kk in range(NT):
                nc.gpsimd.tensor_copy(wo_sb[:, kk, :], wo_stgs[kk][:])

        # ---- attention ----
        # PSUM budget is 8 banks: qk(2) + sp(4) + one of {vpsum(2), up(2)}
        # — vpsum's scope closes before upsum opens.
        with tc.tile_pool(name="qkpsum", bufs=2, space="PSUM") as qkpsum, \
             tc.tile_pool(name="spsum", bufs=2, space="PSUM") as spsum:

            def qk_proj_thunks(hp, w_cols):
                """Yield thunks: 32 matmuls + 4 bias evacs for pair hp."""
                qt_n = qkpool.tile([P, S], BF16, tag="qt", name=f"qt{hp}")
                kt_n = qkpool.tile([P, S], BF16, tag="kt", name=f"kt{hp}")
                thunks = []
                for w_t, dst, b_sb in ((w_cols[0], qt_n, bq_sb),
                                       (w_cols[1], kt_n, bk_sb)):
                    for n in range(2):
                        def group(w_t=w_t, dst=dst, b_sb=b_sb, n=n):
                            qps = qkpsum.tile([P, 512], F32, tag="qk",
                                              name=f"qk{hp}_{n}")
                            for kk in range(NT):
                                def mm(qps=qps, w_t=w_t, kk=kk, n=n):
                                    nc.tensor.matmul(
                                        qps[:],
                                        w_t[:, kk, :],
                                        xT[:, kk, 512 * n:512 * (n + 1)],
                                        start=(kk == 0), stop=(kk == NT - 1))
                                yield mm
                            def bias(qps=qps, dst=dst, b_sb=b_sb, n=n):
                                nc.vector.tensor_scalar_add(
                                    dst[:, 512 * n:512 * (n + 1)], qps[:],
                                    b_sb[:, hp:hp + 1])
                            yield bias
                        thunks.extend(group())
                return qt_n, kt_n, thunks

            def v_proj_half(vpsum, m, n):
                ps = vpsum.tile([P, 512], F32, tag="ppv", name=f"ppv{m}_{n}")
                for kk in range(NT):
                    nc.tensor.matmul(
                        ps[:],
                        xT[:, kk, P * m:P * (m + 1)],
                        wv_sb[:, kk, 512 * n:512 * (n + 1)],
                        start=(kk == 0), stop=(kk == NT - 1))
                vview = (vp[:, m, (D + 1) * 8 * n:(D + 1) * 8 * (n + 1)]
                         .rearrange("p (h d) -> p h d", d=D + 1))
                nc.vector.tensor_add(
                    vview[:, :, 0:D],
                    ps[:].rearrange("p (h d) -> p h d", d=D),
                    bv_bc[:, 512 * n:512 * (n + 1)]
                    .rearrange("p (h d) -> p h d", d=D))
                nc.vector.tensor_copy(vview[:, :, D:D + 1],
                                      ones16[:, 8 * n:8 * (n + 1)]
                                      .unsqueeze(2))

            def scores_exp(hp, qt_c, kt_c, qh, kk):
                sph = spsum.tile([P, 1024], F32, tag="sp",
                                 name=f"sp{hp}_{qh}_{kk}")
                nc.tensor.matmul(
                    sph[:, 0:512],
                    kt_c[0:D, P * kk:P * (kk + 1)],
                    qt_c[0:D, 512 * qh:512 * (qh + 1)],
                    start=True, stop=True)
                nc.tensor.matmul(
                    sph[:, 512:1024],
                    kt_c[D:P, P * kk:P * (kk + 1)],
                    qt_c[D:P, 512 * qh:512 * (qh + 1)],
                    start=True, stop=True)
                pth = ptpool.tile([P, 1024], BF16, tag="pt",
                                  name=f"pt{hp}_{qh}_{kk}")
                nc.scalar.activation(pth[:], sph[:], EXP, scale=SCALE)
                return pth

            def normalize_half(hp, r_e, r_o, qh):
                lo, hi = 512 * qh, 512 * (qh + 1)
                nc.sync.dma_start(rscr[2 * hp:2 * hp + 1, lo:hi],
                                  r_e[0:1, lo:hi])
                nc.sync.dma_start(rscr[2 * hp + 1:2 * hp + 2, lo:hi],
                                  r_o[0:1, lo:hi])
                rb = rbpool.tile([P, 512], F32, tag=f"rb{qh}",
                                 name=f"rb{hp}_{qh}")
                rbi = rbpool.tile([P, 512], F32, tag=f"rbi{qh}",
                                  name=f"rbi{hp}_{qh}")
                nc.sync.dma_start(
                    rb[0:D, :],
                    rscr[2 * hp, lo:hi][None, :].to_broadcast((D, 512)))
                nc.sync.dma_start(
                    rb[D:P, :],
                    rscr[2 * hp + 1, lo:hi][None, :].to_broadcast((D, 512)))
                nc.vector.reciprocal_approx_fast(rbi[:], rb[:])
                nc.vector.tensor_mul(mergedT[:, hp, lo:hi],
                                     mergedT[:, hp, lo:hi], rbi[:])

            class Pair:
                def __init__(self, hp):
                    self.hp = hp
                    self.he, self.ho = 2 * hp, 2 * hp + 1
                    self.r_e = rppool.tile([1, S], F32, tag="rpe",
                                           name=f"rpe{hp}")
                    self.r_o = rppool.tile([1, S], F32, tag="rpo",
                                           name=f"rpo{hp}")
                    self.up = {}

                def alloc_up(self, qh):
                    self.up[qh] = (
                        upsum.tile([D + 1, 512], F32, tag="upe",
                                   name=f"upe{self.hp}_{qh}"),
                        upsum.tile([D + 1, 512], F32, tag="upo",
                                   name=f"upo{self.hp}_{qh}"))

                def u_mms(self, pt, kk, qh):
                    up_e, up_o = self.up[qh]
                    nc.tensor.matmul(
                        up_e[:],
                        vp[:, kk, (D + 1) * self.he:(D + 1) * (self.he + 1)],
                        pt[:, 0:512],
                        start=(kk == 0), stop=(kk == NT - 1))
                    nc.tensor.matmul(
                        up_o[:],
                        vp[:, kk, (D + 1) * self.ho:(D + 1) * (self.ho + 1)],
                        pt[:, 512:1024],
                        start=(kk == 0), stop=(kk == NT - 1))

                def evac(self, qh):
                    hp = self.hp
                    up_e, up_o = self.up[qh]
                    nc.vector.tensor_copy(
                        mergedT[0:D, hp, 512 * qh:512 * (qh + 1)],
                        up_e[0:D, :])
                    nc.vector.tensor_copy(
                        mergedT[D:P, hp, 512 * qh:512 * (qh + 1)],
                        up_o[0:D, :])
                    nc.vector.tensor_copy(
                        self.r_e[0:1, 512 * qh:512 * (qh + 1)],
                        up_e[D:D + 1, :])
                    nc.vector.tensor_copy(
                        self.r_o[0:1, 512 * qh:512 * (qh + 1)],
                        up_o[D:D + 1, :])

                def normalize_half(self, qh):
                    normalize_half(self.hp, self.r_e, self.r_o, qh)

            # -- pair 0: Q/K proj, then ALL scores+exps (u deferred) with
            # pair-1's proj dripped into the exp-paced stream; V-proj runs
            # after, then pair-0's u batch --
            qt_cur, kt_cur, th0 = qk_proj_thunks(0, wqk0)
            for t in th0:
                t()

            qt_nxt, kt_nxt, pending = qk_proj_thunks(1, wqk1)
            pending = list(pending)
            p0 = Pair(0)
            pts0 = []
            for qh in range(2):
                for kk in range(NT):
                    pts0.append(scores_exp(0, qt_cur, kt_cur, qh, kk))
                    for _ in range(3):
                        if pending:
                            pending.pop(0)()
            # Wv casts here: needed first by v_proj below, and late enough
            # in the DVE FIFO not to block pair-0's bias evacs
            for kk in range(NT):
                nc.vector.tensor_copy(wv_sb[:, kk, :], wv_stgs[kk][:])
            with tc.tile_pool(name="vpsum", bufs=2, space="PSUM") as vpsum:
                for m in range(NT):
                    for n in range(2):
                        v_proj_half(vpsum, m, n)
                        if pending:
                            pending.pop(0)()
            with tc.tile_pool(name="upsum", bufs=1, space="PSUM") as upsum_p:
                upsum = upsum_p
                for qh in range(2):
                    p0.alloc_up(qh)
                    for kk in range(NT):
                        p0.u_mms(pts0[qh * NT + kk], kk, qh)
                        if pending:
                            pending.pop(0)()
                    p0.evac(qh)
                    p0.normalize_half(qh)
                while pending:
                    pending.pop(0)()
                qt_cur, kt_cur = qt_nxt, kt_nxt

                # -- pairs 1..7: flat 16-slot loop, u lagged 2 slots; the
                # last 2 u units + evac + normalize of pair p carry into
                # pair p+1's first slots (no PE tail before next scores) --
                carry = None
                for hp in range(1, NT):
                    pending = []
                    if hp + 1 < NT:
                        w_cols = load_wqk_cols(hp + 1)
                        qt_nxt, kt_nxt, pending = qk_proj_thunks(
                            hp + 1, w_cols)
                        pending = list(pending)

                    pr = Pair(hp)
                    pr.alloc_up(0)
                    pts = []
                    for s in range(NT * 2):
                        qh, kk = divmod(s, NT)
                        pts.append(scores_exp(hp, qt_cur, kt_cur, qh, kk))
                        if carry is not None:
                            cpr, cpts = carry
                            if s == 0:
                                cpr.u_mms(cpts[14], NT - 2, 1)
                            elif s == 1:
                                cpr.u_mms(cpts[15], NT - 1, 1)
                                cpr.evac(1)
                                cpr.normalize_half(1)
                                carry = None
                        if s == 2 + NT:
                            pr.alloc_up(1)
                        if 2 <= s < 2 + 14:
                            lqh, lkk = divmod(s - 2, NT)
                            pr.u_mms(pts[s - 2], lkk, lqh)
                            if s - 2 == NT - 1:
                                pr.evac(0)
                                pr.normalize_half(0)
                        for _ in range(3):
                            if pending:
                                pending.pop(0)()
                    while pending:
                        pending.pop(0)()
                    if hp + 1 < NT:
                        carry = (pr, pts)
                        qt_cur, kt_cur = qt_nxt, kt_nxt
                    else:
                        pr.u_mms(pts[14], NT - 2, 1)
                        pr.u_mms(pts[15], NT - 1, 1)
                        pr.evac(1)
                        pr.normalize_half(1)

        # ---- output projection ----
        with tc.tile_pool(name="opsum", bufs=4, space="PSUM") as opsum, \
             tc.tile_pool(name="ostage", bufs=4) as ostage:
            for q in range(NT):
                for n in range(2):
                    ps = opsum.tile([P, 512], F32, tag="op", name=f"op{q}_{n}")
                    for r in range(NT):
                        nc.tensor.matmul(
                            ps[:],
                            mergedT[:, r, P * q:P * (q + 1)],
                            wo_sb[:, r, 512 * n:512 * (n + 1)],
                            start=(r == 0), stop=(r == NT - 1))
                    os_t = ostage.tile([P, 512], F32, tag="os",
                                       name=f"os{q}_{n}")
                    nc.vector.tensor_add(os_t[:], ps[:],
                                         bo_bc[:, 512 * n:512 * (n + 1)])
                    nc.sync.dma_start(
                        out[P * q:P * (q + 1), 512 * n:512 * (n + 1)], os_t[:])

    nc.finalize()
    return nc


def _in_maps(inputs):
    x = np.ascontiguousarray(np.asarray(inputs["x"], dtype=np.float32))
    eye = np.eye(P, dtype=np.float32)
    common = {k: np.ascontiguousarray(np.asarray(inputs[k], dtype=np.float32))
              for k in ("Wq", "Wk", "Wv", "Wo", "bq", "bk", "bv", "bo")}
    return [{"x": x[b], "ident": eye, **common} for b in range(B)]


def _gather(res, inputs):
    return np.stack([res.results[b]["out"] for b in range(B)]).astype(np.float32)


def kernel(**inputs):
    from concourse.bass_utils import run_bass_kernel_spmd

    nc = _CACHE.get("nc")
    if nc is None:
        nc = _CACHE["nc"] = _build()

    in_maps = _in_maps(inputs)
    res = run_bass_kernel_spmd(nc, in_maps, list(range(B)))
    return _gather(res, inputs)


# revision 35
# speedup vs baseline: 1.1584x; 1.0585x over previous
"""CLIP attention (B=8, S=1024, H=1024, 16 heads) on 8 TRN2 NeuronCores.

Sharding: data-parallel over batch — core b computes attention for x[b].

v9 (= v4 structure + wo-early + split normalize):
  - bf16 everywhere on the PE: weights/x DMA f32, cast to bf16 on
    DVE/GPSIMD (strided byte-slice DMA and DMA-xbar transpose both
    measured slower — dead ends)
  - pair 0 runs ALL 16 scores+exp units first (pt pool holds them; u
    deferred) while V-proj + pair-1's Q/K projection run on the PE
    underneath, then pair-0's u batch
  - steady pairs: flat 16-slot loop, u lagged 2 slots behind exp
  - normalization per qh-half right after each evac: shortens the
    pair-7 -> output-projection dependency chain
  - Wo row-chunks staged per pair starting with chunk 0 at pair 1
    (chunk 0 gated the entire output projection when staged last)
  - reciprocal_approx_fast for softmax denominators
"""

import numpy as np

B = 8
S = 1024
H = 1024
NH = 16
D = 64
P = 128
NT = 8          # number of 128-tiles along S or H
SCALE = 0.125   # 1/sqrt(64)

_CACHE = {}


def _build():
    import concourse.bacc as bacc
    import concourse.mybir as mybir
    import concourse.tile as tile
    from contextlib import ExitStack

    F32 = mybir.dt.float32
    F32R = mybir.dt.float32r
    BF16 = mybir.dt.bfloat16
    EXP = mybir.ActivationFunctionType.Exp

    nc = bacc.Bacc(None)
    x = nc.dram_tensor("x", [S, H], F32, kind="ExternalInput")
    wq = nc.dram_tensor("Wq", [H, H], F32, kind="ExternalInput")
    wk = nc.dram_tensor("Wk", [H, H], F32, kind="ExternalInput")
    wv = nc.dram_tensor("Wv", [H, H], F32, kind="ExternalInput")
    wo = nc.dram_tensor("Wo", [H, H], F32, kind="ExternalInput")
    bq = nc.dram_tensor("bq", [H], F32, kind="ExternalInput")
    bk = nc.dram_tensor("bk", [H], F32, kind="ExternalInput")
    bv = nc.dram_tensor("bv", [H], F32, kind="ExternalInput")
    bo = nc.dram_tensor("bo", [H], F32, kind="ExternalInput")
    ident = nc.dram_tensor("ident", [P, P], F32, kind="ExternalInput")
    out = nc.dram_tensor("out", [S, H], F32, kind="ExternalOutput")
    rscr = nc.dram_tensor("rscr", [NH, S], F32)   # scratch for r broadcast

    with tile.TileContext(nc) as tc, ExitStack() as ctx:
        pers = ctx.enter_context(tc.tile_pool(name="pers", bufs=1))
        xT = pers.tile([P, NT, S], BF16, name="xT")
        vp = pers.tile([P, NT, NH * (D + 1)], BF16, name="vp")
        mergedT = pers.tile([P, NT, S], BF16, name="mergedT")
        wo_sb = pers.tile([P, NT, H], BF16, name="wo_sb")
        wv_sb = pers.tile([P, NT, H], BF16, name="wv_sb")

        small = ctx.enter_context(tc.tile_pool(name="small", bufs=1))
        bq_sb = small.tile([P, NT], F32, name="bq_sb")
        bk_sb = small.tile([P, NT], F32, name="bk_sb")
        bv_bc = small.tile([P, H], F32, name="bv_bc")
        bo_bc = small.tile([P, H], F32, name="bo_bc")
        ones16 = small.tile([P, NH], F32, name="ones16")

        wqkstage = ctx.enter_context(tc.tile_pool(name="wqks", bufs=2))
        wqkpool = ctx.enter_context(tc.tile_pool(name="wqk", bufs=2))
        wostage = ctx.enter_context(tc.tile_pool(name="wost", bufs=2))
        wvstage = ctx.enter_context(tc.tile_pool(name="wvst", bufs=3))
        qkpool = ctx.enter_context(tc.tile_pool(name="qk", bufs=2))
        ptpool = ctx.enter_context(tc.tile_pool(name="ptp", bufs=16))
        rbpool = ctx.enter_context(tc.tile_pool(name="rb", bufs=1))
        rppool = ctx.enter_context(tc.tile_pool(name="rp", bufs=1))

        def load_wqk_cols(hp):
            """DMA the [H, 128] column slice of Wq/Wk for pair hp (f32)
            and DVE-cast into bf16 [128, NT, 128] tiles."""
            tiles = []
            for src, nm in ((wq, "q"), (wk, "k")):
                stg = wqkstage.tile([P, NT, P], F32, tag=f"ws{nm}",
                                    name=f"ws{nm}{hp}")
                nc.sync.dma_start(
                    stg[:],
                    src[:, P * hp:P * (hp + 1)]
                    .rearrange("(kk p) c -> p kk c", p=P))
                t = wqkpool.tile([P, NT, P], BF16, tag=f"w{nm}c",
                                 name=f"w{nm}c{hp}")
                nc.vector.tensor_copy(t[:], stg[:])
                tiles.append(t)
            return tiles

        # ---- phase 0: DMAs + x transposes + pair-0 Q/K projection ----
        with tc.tile_pool(name="xstage", bufs=3) as xstage, \
             tc.tile_pool(name="idpool", bufs=1) as idpool, \
             tc.tile_pool(name="tpsum", bufs=4, space="PSUM") as tpsum:
            identity = idpool.tile([P, P], F32R, name="identity")
            nc.sync.dma_start(identity[:], ident[:, :].bitcast(F32R))

            xs_tiles = []
            for m in range(NT):
                xs = xstage.tile([P, H], F32R, tag="xs", name=f"xs{m}")
                nc.sync.dma_start(xs[:], x[P * m:P * (m + 1), :].bitcast(F32R))
                xs_tiles.append(xs)
            wqk0 = load_wqk_cols(0)
            nc.sync.dma_start(bq_sb[:], bq.rearrange("(r p) -> p r", p=P))
            nc.sync.dma_start(bk_sb[:], bk.rearrange("(r p) -> p r", p=P))
            nc.sync.dma_start(bv_bc[:], bv[None, :].to_broadcast((P, H)))
            nc.vector.memset(ones16[:], 1.0)
            # Wv row-chunks f32 -> DVE cast (issued after x + pair-0 cols)
            for kk in range(NT):
                ws = wvstage.tile([P, H], F32, tag="wvs", name=f"wvs{kk}")
                nc.sync.dma_start(ws[:], wv[P * kk:P * (kk + 1), :])
                nc.vector.tensor_copy(wv_sb[:, kk, :], ws[:])

            for m in range(NT):
                xs = xs_tiles[m]
                for r in range(NT):
                    tp = tpsum.tile([P, P], F32R, tag="tp", name=f"tp{m}_{r}")
                    nc.tensor.transpose(tp[:], xs[:, P * r:P * (r + 1)],
                                        identity[:])
                    # alternate evac engine: ACT is idle in this phase
                    if r % 2 == 0:
                        nc.vector.tensor_copy(xT[:, r, P * m:P * (m + 1)],
                                              tp[:].bitcast(F32))
                    else:
                        nc.scalar.copy(xT[:, r, P * m:P * (m + 1)],
                                       tp[:].bitcast(F32))

        # ---- attention ----
        # PSUM budget is 8 banks: qk(2) + sp(4) + one of {vpsum(2), up(2)}
        # — vpsum's scope closes before upsum opens.
        with tc.tile_pool(name="qkpsum", bufs=2, space="PSUM") as qkpsum, \
             tc.tile_pool(name="spsum", bufs=2, space="PSUM") as spsum:

            def qk_proj_thunks(hp, w_cols):
                """Yield thunks: 32 matmuls + 4 bias evacs for pair hp."""
                qt_n = qkpool.tile([P, S], BF16, tag="qt", name=f"qt{hp}")
                kt_n = qkpool.tile([P, S], BF16, tag="kt", name=f"kt{hp}")
                thunks = []
                for w_t, dst, b_sb in ((w_cols[0], qt_n, bq_sb),
                                       (w_cols[1], kt_n, bk_sb)):
                    for n in range(2):
                        def group(w_t=w_t, dst=dst, b_sb=b_sb, n=n):
                            qps = qkpsum.tile([P, 512], F32, tag="qk",
                                              name=f"qk{hp}_{n}")
                            for kk in range(NT):
                                def mm(qps=qps, w_t=w_t, kk=kk, n=n):
                                    nc.tensor.matmul(
                                        qps[:],
                                        w_t[:, kk, :],
                                        xT[:, kk, 512 * n:512 * (n + 1)],
                                        start=(kk == 0), stop=(kk == NT - 1))
                                yield mm
                            def bias(qps=qps, dst=dst, b_sb=b_sb, n=n):
                                nc.vector.tensor_scalar_add(
                                    dst[:, 512 * n:512 * (n + 1)], qps[:],
                                    b_sb[:, hp:hp + 1])
                            yield bias
                        thunks.extend(group())
                return qt_n, kt_n, thunks

            def v_proj_half(vpsum, m, n):
                ps = vpsum.tile([P, 512], F32, tag="ppv", name=f"ppv{m}_{n}")
                for kk in range(NT):
                    nc.tensor.matmul(
                        ps[:],
                        xT[:, kk, P * m:P * (m + 1)],
                        wv_sb[:, kk, 512 * n:512 * (n + 1)],
                        start=(kk == 0), stop=(kk == NT - 1))
                vview = (vp[:, m, (D + 1) * 8 * n:(D + 1) * 8 * (n + 1)]
                         .rearrange("p (h d) -> p h d", d=D + 1))
                nc.vector.tensor_add(
                    vview[:, :, 0:D],
                    ps[:].rearrange("p (h d) -> p h d", d=D),
                    bv_bc[:, 512 * n:512 * (n + 1)]
                    .rearrange("p (h d) -> p h d", d=D))
                nc.vector.tensor_copy(vview[:, :, D:D + 1],
                                      ones16[:, 8 * n:8 * (n + 1)]
                                      .unsqueeze(2))

            def scores_exp(hp, qt_c, kt_c, qh, kk):
                sph = spsum.tile([P, 1024], F32, tag="sp",
                                 name=f"sp{hp}_{qh}_{kk}")
                nc.tensor.matmul(
                    sph[:, 0:512],
                    kt_c[0:D, P * kk:P * (kk + 1)],
                    qt_c[0:D, 512 * qh:512 * (qh + 1)],
                    start=True, stop=True)
                nc.tensor.matmul(
                    sph[:, 512:1024],
                    kt_c[D:P, P * kk:P * (kk + 1)],
                    qt_c[D:P, 512 * qh:512 * (qh + 1)],
                    start=True, stop=True)
                pth = ptpool.tile([P, 1024], BF16, tag="pt",
                                  name=f"pt{hp}_{qh}_{kk}")
                nc.scalar.activation(pth[:], sph[:], EXP, scale=SCALE)
                return pth

            def normalize_half(hp, r_e, r_o, qh):
                lo, hi = 512 * qh, 512 * (qh + 1)
                nc.sync.dma_start(rscr[2 * hp:2 * hp + 1, lo:hi],
                                  r_e[0:1, lo:hi])
                nc.sync.dma_start(rscr[2 * hp + 1:2 * hp + 2, lo:hi],
                                  r_o[0:1, lo:hi])
                rb = rbpool.tile([P, 512], F32, tag=f"rb{qh}",
                                 name=f"rb{hp}_{qh}")
                rbi = rbpool.tile([P, 512], F32, tag=f"rbi{qh}",
                                  name=f"rbi{hp}_{qh}")
                nc.sync.dma_start(
                    rb[0:D, :],
                    rscr[2 * hp, lo:hi][None, :].to_broadcast((D, 512)))
                nc.sync.dma_start(
                    rb[D:P, :],
                    rscr[2 * hp + 1, lo:hi][None, :].to_broadcast((D, 512)))
                nc.vector.reciprocal_approx_fast(rbi[:], rb[:])
                nc.vector.tensor_mul(mergedT[:, hp, lo:hi],
                                     mergedT[:, hp, lo:hi], rbi[:])

            class Pair:
                def __init__(self, hp):
                    self.hp = hp
                    self.he, self.ho = 2 * hp, 2 * hp + 1
                    self.r_e = rppool.tile([1, S], F32, tag="rpe",
                                           name=f"rpe{hp}")
                    self.r_o = rppool.tile([1, S], F32, tag="rpo",
                                           name=f"rpo{hp}")
                    self.up = {}

                def alloc_up(self, qh):
                    self.up[qh] = (
                        upsum.tile([D + 1, 512], F32, tag="upe",
                                   name=f"upe{self.hp}_{qh}"),
                        upsum.tile([D + 1, 512], F32, tag="upo",
                                   name=f"upo{self.hp}_{qh}"))

                def u_mms(self, pt, kk, qh):
                    up_e, up_o = self.up[qh]
                    nc.tensor.matmul(
                        up_e[:],
                        vp[:, kk, (D + 1) * self.he:(D + 1) * (self.he + 1)],
                        pt[:, 0:512],
                        start=(kk == 0), stop=(kk == NT - 1))
                    nc.tensor.matmul(
                        up_o[:],
                        vp[:, kk, (D + 1) * self.ho:(D + 1) * (self.ho + 1)],
                        pt[:, 512:1024],
                        start=(kk == 0), stop=(kk == NT - 1))

                def evac(self, qh):
                    hp = self.hp
                    up_e, up_o = self.up[qh]
                    nc.vector.tensor_copy(
                        mergedT[0:D, hp, 512 * qh:512 * (qh + 1)],
                        up_e[0:D, :])
                    nc.vector.tensor_copy(
                        mergedT[D:P, hp, 512 * qh:512 * (qh + 1)],
                        up_o[0:D, :])
                    nc.vector.tensor_copy(
                        self.r_e[0:1, 512 * qh:512 * (qh + 1)],
                        up_e[D:D + 1, :])
                    nc.vector.tensor_copy(
                        self.r_o[0:1, 512 * qh:512 * (qh + 1)],
                        up_o[D:D + 1, :])

                def normalize_half(self, qh):
                    normalize_half(self.hp, self.r_e, self.r_o, qh)

            # -- pair 0: Q/K proj, then ALL scores+exps (u deferred), with
            # V-proj + pair-1 proj on the PE under the exp stream --
            qt_cur, kt_cur, th0 = qk_proj_thunks(0, wqk0)
            for t in th0:
                t()

            p0 = Pair(0)
            pts0 = []
            for qh in range(2):
                for kk in range(NT):
                    pts0.append(scores_exp(0, qt_cur, kt_cur, qh, kk))

            w_cols1 = load_wqk_cols(1)
            qt_nxt, kt_nxt, pending = qk_proj_thunks(1, w_cols1)
            pending = list(pending)
            with tc.tile_pool(name="vpsum", bufs=2, space="PSUM") as vpsum:
                for m in range(NT):
                    for n in range(2):
                        v_proj_half(vpsum, m, n)
                        for _ in range(2):
                            if pending:
                                pending.pop(0)()
            with tc.tile_pool(name="upsum", bufs=1, space="PSUM") as upsum_p:
                upsum = upsum_p
                for qh in range(2):
                    p0.alloc_up(qh)
                    for kk in range(NT):
                        p0.u_mms(pts0[qh * NT + kk], kk, qh)
                        if pending:
                            pending.pop(0)()
                    p0.evac(qh)
                    p0.normalize_half(qh)
                while pending:
                    pending.pop(0)()
                qt_cur, kt_cur = qt_nxt, kt_nxt

                # -- pairs 1..7: flat 16-slot loop, u lagged 2 slots --
                for hp in range(1, NT):
                    pending = []
                    if hp + 1 < NT:
                        w_cols = load_wqk_cols(hp + 1)
                        qt_nxt, kt_nxt, pending = qk_proj_thunks(
                            hp + 1, w_cols)
                        pending = list(pending)
                    # stage wo row-chunks hp-1 (and 7 at the last pair):
                    # chunk 0 is the FIRST accumulation step of every
                    # output-projection tile — it must not arrive last
                    chunks = [hp - 1] + ([NT - 1] if hp == NT - 1 else [])
                    for ck in chunks:
                        wos = wostage.tile([P, H], F32, tag="wos",
                                           name=f"wos{ck}")
                        nc.sync.dma_start(wos[:], wo[P * ck:P * (ck + 1), :])
                        nc.gpsimd.tensor_copy(wo_sb[:, ck, :], wos[:])

                    pr = Pair(hp)
                    pr.alloc_up(0)
                    pts = []
                    for s in range(NT * 2 + 2):
                        if s < NT * 2:
                            qh, kk = divmod(s, NT)
                            pts.append(scores_exp(hp, qt_cur, kt_cur, qh, kk))
                        if s == 2 + NT:
                            pr.alloc_up(1)
                        if s >= 2:
                            lqh, lkk = divmod(s - 2, NT)
                            pr.u_mms(pts[s - 2], lkk, lqh)
                            if s - 2 == NT - 1:
                                pr.evac(0)
                                pr.normalize_half(0)
                            elif s - 2 == 2 * NT - 1:
                                pr.evac(1)
                                pr.normalize_half(1)
                        for _ in range(3):
                            if pending:
                                pending.pop(0)()
                    while pending:
                        pending.pop(0)()
                    if hp + 1 < NT:
                        qt_cur, kt_cur = qt_nxt, kt_nxt

        nc.sync.dma_start(bo_bc[:], bo[None, :].to_broadcast((P, H)))

        # ---- output projection ----
        with tc.tile_pool(name="opsum", bufs=4, space="PSUM") as opsum, \
             tc.tile_pool(name="ostage", bufs=4) as ostage:
            for q in range(NT):
                for n in range(2):
                    ps = opsum.tile([P, 512], F32, tag="op", name=f"op{q}_{n}")
                    for r in range(NT):
                        nc.tensor.matmul(
                            ps[:],
                            mergedT[:, r, P * q:P * (q + 1)],
                            wo_sb[:, r, 512 * n:512 * (n + 1)],
                            start=(r == 0), stop=(r == NT - 1))
                    os_t = ostage.tile([P, 512], F32, tag="os",
                                       name=f"os{q}_{n}")
                    nc.vector.tensor_add(os_t[:], ps[:],
                                         bo_bc[:, 512 * n:512 * (n + 1)])
                    nc.sync.dma_start(
                        out[P * q:P * (q + 1), 512 * n:512 * (n + 1)], os_t[:])

    nc.finalize()
    return nc


def _in_maps(inputs):
    x = np.ascontiguousarray(np.asarray(inputs["x"], dtype=np.float32))
    eye = np.eye(P, dtype=np.float32)
    common = {k: np.ascontiguousarray(np.asarray(inputs[k], dtype=np.float32))
              for k in ("Wq", "Wk", "Wv", "Wo", "bq", "bk", "bv", "bo")}
    return [{"x": x[b], "ident": eye, **common} for b in range(B)]


def _gather(res, inputs):
    return np.stack([res.results[b]["out"] for b in range(B)]).astype(np.float32)


def kernel(**inputs):
    from concourse.bass_utils import run_bass_kernel_spmd

    nc = _CACHE.get("nc")
    if nc is None:
        nc = _CACHE["nc"] = _build()

    in_maps = _in_maps(inputs)
    res = run_bass_kernel_spmd(nc, in_maps, list(range(B)))
    return _gather(res, inputs)


# revision 40
# speedup vs baseline: 1.1694x; 1.0095x over previous
"""CLIP attention (B=8, S=1024, H=1024, 16 heads) on 8 TRN2 NeuronCores.

Sharding: data-parallel over batch — core b computes attention for x[b].

v9 (= v4 structure + wo-early + split normalize):
  - bf16 everywhere on the PE: weights/x DMA f32, cast to bf16 on
    DVE/GPSIMD (strided byte-slice DMA and DMA-xbar transpose both
    measured slower — dead ends)
  - pair 0 runs ALL 16 scores+exp units first (pt pool holds them; u
    deferred) while V-proj + pair-1's Q/K projection run on the PE
    underneath, then pair-0's u batch
  - steady pairs: flat 16-slot loop, u lagged 2 slots behind exp
  - normalization per qh-half right after each evac: shortens the
    pair-7 -> output-projection dependency chain
  - Wo row-chunks staged per pair starting with chunk 0 at pair 1
    (chunk 0 gated the entire output projection when staged last)
  - reciprocal_approx_fast for softmax denominators
"""

import numpy as np

B = 8
S = 1024
H = 1024
NH = 16
D = 64
P = 128
NT = 8          # number of 128-tiles along S or H
SCALE = 0.125   # 1/sqrt(64)

_CACHE = {}


def _build():
    import concourse.bacc as bacc
    import concourse.mybir as mybir
    import concourse.tile as tile
    from contextlib import ExitStack

    F32 = mybir.dt.float32
    F32R = mybir.dt.float32r
    BF16 = mybir.dt.bfloat16
    EXP = mybir.ActivationFunctionType.Exp

    nc = bacc.Bacc(None)
    x = nc.dram_tensor("x", [S, H], F32, kind="ExternalInput")
    wq = nc.dram_tensor("Wq", [H, H], F32, kind="ExternalInput")
    wk = nc.dram_tensor("Wk", [H, H], F32, kind="ExternalInput")
    wv = nc.dram_tensor("Wv", [H, H], F32, kind="ExternalInput")
    wo = nc.dram_tensor("Wo", [H, H], F32, kind="ExternalInput")
    bq = nc.dram_tensor("bq", [H], F32, kind="ExternalInput")
    bk = nc.dram_tensor("bk", [H], F32, kind="ExternalInput")
    bv = nc.dram_tensor("bv", [H], F32, kind="ExternalInput")
    bo = nc.dram_tensor("bo", [H], F32, kind="ExternalInput")
    ident = nc.dram_tensor("ident", [P, P], F32, kind="ExternalInput")
    out = nc.dram_tensor("out", [S, H], F32, kind="ExternalOutput")
    rscr = nc.dram_tensor("rscr", [NH, S], F32)   # scratch for r broadcast

    with tile.TileContext(nc) as tc, ExitStack() as ctx:
        pers = ctx.enter_context(tc.tile_pool(name="pers", bufs=1))
        xT = pers.tile([P, NT, S], BF16, name="xT")
        vp = pers.tile([P, NT, NH * (D + 1)], BF16, name="vp")
        mergedT = pers.tile([P, NT, S], BF16, name="mergedT")
        wo_sb = pers.tile([P, NT, H], BF16, name="wo_sb")
        wv_sb = pers.tile([P, NT, H], BF16, name="wv_sb")

        small = ctx.enter_context(tc.tile_pool(name="small", bufs=1))
        bq_sb = small.tile([P, NT], F32, name="bq_sb")
        bk_sb = small.tile([P, NT], F32, name="bk_sb")
        bv_bc = small.tile([P, H], F32, name="bv_bc")
        bo_bc = small.tile([P, H], F32, name="bo_bc")
        ones16 = small.tile([P, NH], F32, name="ones16")

        wqkstage = ctx.enter_context(tc.tile_pool(name="wqks", bufs=2))
        wqkpool = ctx.enter_context(tc.tile_pool(name="wqk", bufs=2))
        wostage = ctx.enter_context(tc.tile_pool(name="wost", bufs=2))
        wvstage = ctx.enter_context(tc.tile_pool(name="wvst", bufs=3))
        qkpool = ctx.enter_context(tc.tile_pool(name="qk", bufs=2))
        ptpool = ctx.enter_context(tc.tile_pool(name="ptp", bufs=16))
        rbpool = ctx.enter_context(tc.tile_pool(name="rb", bufs=1))
        rppool = ctx.enter_context(tc.tile_pool(name="rp", bufs=1))

        def load_wqk_cols(hp):
            """DMA the [H, 128] column slice of Wq/Wk for pair hp (f32)
            and DVE-cast into bf16 [128, NT, 128] tiles."""
            tiles = []
            for src, nm in ((wq, "q"), (wk, "k")):
                stg = wqkstage.tile([P, NT, P], F32, tag=f"ws{nm}",
                                    name=f"ws{nm}{hp}")
                nc.sync.dma_start(
                    stg[:],
                    src[:, P * hp:P * (hp + 1)]
                    .rearrange("(kk p) c -> p kk c", p=P))
                t = wqkpool.tile([P, NT, P], BF16, tag=f"w{nm}c",
                                 name=f"w{nm}c{hp}")
                nc.vector.tensor_copy(t[:], stg[:])
                tiles.append(t)
            return tiles

        # ---- phase 0: DMAs + x transposes + pair-0 Q/K projection ----
        with tc.tile_pool(name="xstage", bufs=3) as xstage, \
             tc.tile_pool(name="idpool", bufs=1) as idpool, \
             tc.tile_pool(name="tpsum", bufs=4, space="PSUM") as tpsum:
            identity = idpool.tile([P, P], F32R, name="identity")
            nc.sync.dma_start(identity[:], ident[:, :].bitcast(F32R))

            xs_tiles = []
            for m in range(NT):
                xs = xstage.tile([P, H], F32R, tag="xs", name=f"xs{m}")
                eng = nc.sync if m % 2 == 0 else nc.scalar
                eng.dma_start(xs[:], x[P * m:P * (m + 1), :].bitcast(F32R))
                xs_tiles.append(xs)
            wqk0 = load_wqk_cols(0)
            nc.sync.dma_start(bq_sb[:], bq.rearrange("(r p) -> p r", p=P))
            nc.sync.dma_start(bk_sb[:], bk.rearrange("(r p) -> p r", p=P))
            nc.sync.dma_start(bv_bc[:], bv[None, :].to_broadcast((P, H)))
            nc.vector.memset(ones16[:], 1.0)
            # Wv row-chunks f32 -> DVE cast (issued after x + pair-0 cols)
            for kk in range(NT):
                ws = wvstage.tile([P, H], F32, tag="wvs", name=f"wvs{kk}")
                nc.sync.dma_start(ws[:], wv[P * kk:P * (kk + 1), :])
                nc.vector.tensor_copy(wv_sb[:, kk, :], ws[:])

            for m in range(NT):
                xs = xs_tiles[m]
                for r in range(NT):
                    tp = tpsum.tile([P, P], F32R, tag="tp", name=f"tp{m}_{r}")
                    nc.tensor.transpose(tp[:], xs[:, P * r:P * (r + 1)],
                                        identity[:])
                    # alternate evac engine: ACT is idle in this phase
                    if r % 2 == 0:
                        nc.vector.tensor_copy(xT[:, r, P * m:P * (m + 1)],
                                              tp[:].bitcast(F32))
                    else:
                        nc.scalar.copy(xT[:, r, P * m:P * (m + 1)],
                                       tp[:].bitcast(F32))

        # ---- attention ----
        # PSUM budget is 8 banks: qk(2) + sp(4) + one of {vpsum(2), up(2)}
        # — vpsum's scope closes before upsum opens.
        with tc.tile_pool(name="qkpsum", bufs=2, space="PSUM") as qkpsum, \
             tc.tile_pool(name="spsum", bufs=2, space="PSUM") as spsum:

            def qk_proj_thunks(hp, w_cols):
                """Yield thunks: 32 matmuls + 4 bias evacs for pair hp."""
                qt_n = qkpool.tile([P, S], BF16, tag="qt", name=f"qt{hp}")
                kt_n = qkpool.tile([P, S], BF16, tag="kt", name=f"kt{hp}")
                thunks = []
                for w_t, dst, b_sb in ((w_cols[0], qt_n, bq_sb),
                                       (w_cols[1], kt_n, bk_sb)):
                    for n in range(2):
                        def group(w_t=w_t, dst=dst, b_sb=b_sb, n=n):
                            qps = qkpsum.tile([P, 512], F32, tag="qk",
                                              name=f"qk{hp}_{n}")
                            for kk in range(NT):
                                def mm(qps=qps, w_t=w_t, kk=kk, n=n):
                                    nc.tensor.matmul(
                                        qps[:],
                                        w_t[:, kk, :],
                                        xT[:, kk, 512 * n:512 * (n + 1)],
                                        start=(kk == 0), stop=(kk == NT - 1))
                                yield mm
                            def bias(qps=qps, dst=dst, b_sb=b_sb, n=n):
                                nc.vector.tensor_scalar_add(
                                    dst[:, 512 * n:512 * (n + 1)], qps[:],
                                    b_sb[:, hp:hp + 1])
                            yield bias
                        thunks.extend(group())
                return qt_n, kt_n, thunks

            def v_proj_half(vpsum, m, n):
                ps = vpsum.tile([P, 512], F32, tag="ppv", name=f"ppv{m}_{n}")
                for kk in range(NT):
                    nc.tensor.matmul(
                        ps[:],
                        xT[:, kk, P * m:P * (m + 1)],
                        wv_sb[:, kk, 512 * n:512 * (n + 1)],
                        start=(kk == 0), stop=(kk == NT - 1))
                vview = (vp[:, m, (D + 1) * 8 * n:(D + 1) * 8 * (n + 1)]
                         .rearrange("p (h d) -> p h d", d=D + 1))
                nc.vector.tensor_add(
                    vview[:, :, 0:D],
                    ps[:].rearrange("p (h d) -> p h d", d=D),
                    bv_bc[:, 512 * n:512 * (n + 1)]
                    .rearrange("p (h d) -> p h d", d=D))
                nc.vector.tensor_copy(vview[:, :, D:D + 1],
                                      ones16[:, 8 * n:8 * (n + 1)]
                                      .unsqueeze(2))

            def scores_exp(hp, qt_c, kt_c, qh, kk):
                sph = spsum.tile([P, 1024], F32, tag="sp",
                                 name=f"sp{hp}_{qh}_{kk}")
                nc.tensor.matmul(
                    sph[:, 0:512],
                    kt_c[0:D, P * kk:P * (kk + 1)],
                    qt_c[0:D, 512 * qh:512 * (qh + 1)],
                    start=True, stop=True)
                nc.tensor.matmul(
                    sph[:, 512:1024],
                    kt_c[D:P, P * kk:P * (kk + 1)],
                    qt_c[D:P, 512 * qh:512 * (qh + 1)],
                    start=True, stop=True)
                pth = ptpool.tile([P, 1024], BF16, tag="pt",
                                  name=f"pt{hp}_{qh}_{kk}")
                nc.scalar.activation(pth[:], sph[:], EXP, scale=SCALE)
                return pth

            def normalize_half(hp, r_e, r_o, qh):
                lo, hi = 512 * qh, 512 * (qh + 1)
                nc.sync.dma_start(rscr[2 * hp:2 * hp + 1, lo:hi],
                                  r_e[0:1, lo:hi])
                nc.sync.dma_start(rscr[2 * hp + 1:2 * hp + 2, lo:hi],
                                  r_o[0:1, lo:hi])
                rb = rbpool.tile([P, 512], F32, tag=f"rb{qh}",
                                 name=f"rb{hp}_{qh}")
                rbi = rbpool.tile([P, 512], F32, tag=f"rbi{qh}",
                                  name=f"rbi{hp}_{qh}")
                nc.sync.dma_start(
                    rb[0:D, :],
                    rscr[2 * hp, lo:hi][None, :].to_broadcast((D, 512)))
                nc.sync.dma_start(
                    rb[D:P, :],
                    rscr[2 * hp + 1, lo:hi][None, :].to_broadcast((D, 512)))
                nc.vector.reciprocal_approx_fast(rbi[:], rb[:])
                nc.vector.tensor_mul(mergedT[:, hp, lo:hi],
                                     mergedT[:, hp, lo:hi], rbi[:])

            class Pair:
                def __init__(self, hp):
                    self.hp = hp
                    self.he, self.ho = 2 * hp, 2 * hp + 1
                    self.r_e = rppool.tile([1, S], F32, tag="rpe",
                                           name=f"rpe{hp}")
                    self.r_o = rppool.tile([1, S], F32, tag="rpo",
                                           name=f"rpo{hp}")
                    self.up = {}

                def alloc_up(self, qh):
                    self.up[qh] = (
                        upsum.tile([D + 1, 512], F32, tag="upe",
                                   name=f"upe{self.hp}_{qh}"),
                        upsum.tile([D + 1, 512], F32, tag="upo",
                                   name=f"upo{self.hp}_{qh}"))

                def u_mms(self, pt, kk, qh):
                    up_e, up_o = self.up[qh]
                    nc.tensor.matmul(
                        up_e[:],
                        vp[:, kk, (D + 1) * self.he:(D + 1) * (self.he + 1)],
                        pt[:, 0:512],
                        start=(kk == 0), stop=(kk == NT - 1))
                    nc.tensor.matmul(
                        up_o[:],
                        vp[:, kk, (D + 1) * self.ho:(D + 1) * (self.ho + 1)],
                        pt[:, 512:1024],
                        start=(kk == 0), stop=(kk == NT - 1))

                def evac(self, qh):
                    hp = self.hp
                    up_e, up_o = self.up[qh]
                    nc.vector.tensor_copy(
                        mergedT[0:D, hp, 512 * qh:512 * (qh + 1)],
                        up_e[0:D, :])
                    nc.vector.tensor_copy(
                        mergedT[D:P, hp, 512 * qh:512 * (qh + 1)],
                        up_o[0:D, :])
                    nc.vector.tensor_copy(
                        self.r_e[0:1, 512 * qh:512 * (qh + 1)],
                        up_e[D:D + 1, :])
                    nc.vector.tensor_copy(
                        self.r_o[0:1, 512 * qh:512 * (qh + 1)],
                        up_o[D:D + 1, :])

                def normalize_half(self, qh):
                    normalize_half(self.hp, self.r_e, self.r_o, qh)

            # -- pair 0: Q/K proj, then ALL scores+exps (u deferred), with
            # V-proj + pair-1 proj on the PE under the exp stream --
            qt_cur, kt_cur, th0 = qk_proj_thunks(0, wqk0)
            for t in th0:
                t()

            p0 = Pair(0)
            pts0 = []
            for qh in range(2):
                for kk in range(NT):
                    pts0.append(scores_exp(0, qt_cur, kt_cur, qh, kk))

            w_cols1 = load_wqk_cols(1)
            qt_nxt, kt_nxt, pending = qk_proj_thunks(1, w_cols1)
            pending = list(pending)
            with tc.tile_pool(name="vpsum", bufs=2, space="PSUM") as vpsum:
                for m in range(NT):
                    for n in range(2):
                        v_proj_half(vpsum, m, n)
                        for _ in range(2):
                            if pending:
                                pending.pop(0)()
            with tc.tile_pool(name="upsum", bufs=1, space="PSUM") as upsum_p:
                upsum = upsum_p
                for qh in range(2):
                    p0.alloc_up(qh)
                    for kk in range(NT):
                        p0.u_mms(pts0[qh * NT + kk], kk, qh)
                        if pending:
                            pending.pop(0)()
                    p0.evac(qh)
                    p0.normalize_half(qh)
                while pending:
                    pending.pop(0)()
                qt_cur, kt_cur = qt_nxt, kt_nxt

                # -- pairs 1..7: flat 16-slot loop, u lagged 2 slots --
                for hp in range(1, NT):
                    pending = []
                    if hp + 1 < NT:
                        w_cols = load_wqk_cols(hp + 1)
                        qt_nxt, kt_nxt, pending = qk_proj_thunks(
                            hp + 1, w_cols)
                        pending = list(pending)
                    # stage wo row-chunks hp-1 (and 7 at the last pair):
                    # chunk 0 is the FIRST accumulation step of every
                    # output-projection tile — it must not arrive last
                    chunks = [hp - 1] + ([NT - 1] if hp == NT - 1 else [])
                    for ck in chunks:
                        wos = wostage.tile([P, H], F32, tag="wos",
                                           name=f"wos{ck}")
                        nc.sync.dma_start(wos[:], wo[P * ck:P * (ck + 1), :])
                        nc.gpsimd.tensor_copy(wo_sb[:, ck, :], wos[:])

                    pr = Pair(hp)
                    pr.alloc_up(0)
                    pts = []
                    for s in range(NT * 2 + 2):
                        if s < NT * 2:
                            qh, kk = divmod(s, NT)
                            pts.append(scores_exp(hp, qt_cur, kt_cur, qh, kk))
                        if s == 2 + NT:
                            pr.alloc_up(1)
                        if s >= 2:
                            lqh, lkk = divmod(s - 2, NT)
                            pr.u_mms(pts[s - 2], lkk, lqh)
                            if s - 2 == NT - 1:
                                pr.evac(0)
                                pr.normalize_half(0)
                            elif s - 2 == 2 * NT - 1:
                                pr.evac(1)
                                pr.normalize_half(1)
                        for _ in range(3):
                            if pending:
                                pending.pop(0)()
                    while pending:
                        pending.pop(0)()
                    if hp + 1 < NT:
                        qt_cur, kt_cur = qt_nxt, kt_nxt

        nc.sync.dma_start(bo_bc[:], bo[None, :].to_broadcast((P, H)))

        # ---- output projection ----
        with tc.tile_pool(name="opsum", bufs=4, space="PSUM") as opsum, \
             tc.tile_pool(name="ostage", bufs=4) as ostage:
            for q in range(NT):
                for n in range(2):
                    ps = opsum.tile([P, 512], F32, tag="op", name=f"op{q}_{n}")
                    for r in range(NT):
                        nc.tensor.matmul(
                            ps[:],
                            mergedT[:, r, P * q:P * (q + 1)],
                            wo_sb[:, r, 512 * n:512 * (n + 1)],
                            start=(r == 0), stop=(r == NT - 1))
                    os_t = ostage.tile([P, 512], F32, tag="os",
                                       name=f"os{q}_{n}")
                    nc.vector.tensor_add(os_t[:], ps[:],
                                         bo_bc[:, 512 * n:512 * (n + 1)])
                    nc.sync.dma_start(
                        out[P * q:P * (q + 1), 512 * n:512 * (n + 1)], os_t[:])

    nc.finalize()
    return nc


def _in_maps(inputs):
    x = np.ascontiguousarray(np.asarray(inputs["x"], dtype=np.float32))
    eye = np.eye(P, dtype=np.float32)
    common = {k: np.ascontiguousarray(np.asarray(inputs[k], dtype=np.float32))
              for k in ("Wq", "Wk", "Wv", "Wo", "bq", "bk", "bv", "bo")}
    return [{"x": x[b], "ident": eye, **common} for b in range(B)]


def _gather(res, inputs):
    return np.stack([res.results[b]["out"] for b in range(B)]).astype(np.float32)


def kernel(**inputs):
    from concourse.bass_utils import run_bass_kernel_spmd

    nc = _CACHE.get("nc")
    if nc is None:
        nc = _CACHE["nc"] = _build()

    in_maps = _in_maps(inputs)
    res = run_bass_kernel_spmd(nc, in_maps, list(range(B)))
    return _gather(res, inputs)
